# revision 4
# baseline (speedup 1.0000x reference)
"""AWPINN wavelet-PINN kernel for 8x Trainium2 NeuronCores (Bass/Tile).

Math: for each point i and wavelet k (N=65536, K=512):
  xt = wx*x - bx (same y,z);  s = xt^2+yt^2+zt^2;  E = exp(-0.5*s)
  W  = xt*yt*zt*E          (reference's xw*yw*zw = -W)
  output = sum_k (-coeff*scale)_k * W + bias
  d2u_dx2 = sum_k (coeff*scale*wx^2)_k * (3 - xt^2) * W   (same y,z)

Device structure:
  - s and T3=xt*yt*zt are low-rank bilinear forms in per-point features
    F = [x2,y2,z2,xyz,xy,xz,yz,x,y,z,1] -> TensorEngine matmuls
    (contraction = features, M = 128 wavelets/block, FD = 512 points).
  - All matmuls fp16 (1 cycle/column on PE; fp32/fp32r are ~3x slower).
    Near-fp32 precision via hi/lo splits stacked along the contraction dim:
    [Lh;Ll;Lh] @ [Fh;Fh;Fl] == L @ F with ~2^-21 products. The constant
    feature's lo-row is dropped -> exactly 32 rows, so the four feature
    matmuls of a k-block pair rotate over the PE's four 32-row groups
    (tile_position) and their weight loads overlap the previous matmul.
  - d2 terms decompose via xt^2 = wx^2*x^2 - 2*wx*bx*x + bx^2 into 3
    matvec columns each -> one [128k, 10] output matmul (hi + lo lhsT)
    per k-block, combined per-point in a tiny [128,64] epilogue.
  - k-blocks are processed in pairs; exp (ACT) and W=T3*E (DVE) run at
    FD=1024 over both psum banks of a pair to amortize instruction cost.
Data parallel over points: each core handles 8192 points; no collectives.
"""

import numpy as np

N_TOTAL = 65536
K_TOTAL = 512
N_CORES = 8
NP_CORE = N_TOTAL // N_CORES        # 8192 points per core
CHUNK = 512                         # points per matmul (PSUM bank = 512 fp32)
N_CHUNKS = NP_CORE // CHUNK         # 16
KBLK = K_TOTAL // 128               # 4 wavelet blocks of 128
EPP = NP_CORE // 128                # 64 = free dim of [128, 64] point layout
NFEAT = 11                          # features per point
NST = 32                            # stacked contraction rows (ones-lo dropped)
LO_SPLIT = True                     # hi+lo split of output-matmul lhsT
PACK_FEATURES = True                # tile_position row-group rotation

_COMPILED = {}


def _split16(a):
    """Split fp32 into fp16 hi + fp16 lo (hi+lo carries ~21 mantissa bits)."""
    a = np.ascontiguousarray(a, np.float32)
    hi = a.astype(np.float16)
    lo = np.float32(a - hi.astype(np.float32)).astype(np.float16)
    return hi, lo


def _stack32(L):
    """[11,n] fp32 coeffs -> [32,n] fp16 stack [Lh; Ll; Lh[:10]]."""
    Lh, Ll = _split16(L)
    return np.concatenate([Lh, Ll, Lh[:NFEAT - 1]], axis=0)


def _build_program():
    import concourse.bacc as bacc
    import concourse.mybir as mybir
    import concourse.tile as tile

    f32 = mybir.dt.float32
    f16 = mybir.dt.float16
    AF = mybir.ActivationFunctionType

    nc = bacc.Bacc("TRN2", target_bir_lowering=False, debug=False)

    # fst: feature stack replicated at partition offsets 0/32/64/96
    fst_d = nc.dram_tensor("fst", [4 * NST, NP_CORE], f16, kind="ExternalInput")
    # lst: rows 0-31 Ls-stack, 32-63 Lt-stack, 64-95 Ls, 96-127 Lt;
    # columns grouped by k-block
    lst_d = nc.dram_tensor("lst", [128, K_TOTAL], f16, kind="ExternalInput")
    loh_d = nc.dram_tensor("loh", [128, KBLK * 10], f16, kind="ExternalInput")
    lol_d = nc.dram_tensor("lol", [128, KBLK * 10], f16, kind="ExternalInput")
    ep_d = nc.dram_tensor("ep", [6, NP_CORE], f32, kind="ExternalInput")
    out_d = nc.dram_tensor("out", [4, NP_CORE], f32, kind="ExternalOutput")

    with tile.TileContext(nc) as tc:
        with (
            tc.tile_pool(name="persist", bufs=1) as pp,
            tc.tile_pool(name="fpool", bufs=3) as fpool,
            tc.tile_pool(name="work", bufs=3) as wp,
            tc.tile_pool(name="psum_s", bufs=2, space="PSUM") as psps,
            tc.tile_pool(name="psum_t", bufs=1, space="PSUM") as pspt,
            tc.tile_pool(name="psum_out", bufs=2, space="PSUM") as pso,
            tc.tile_pool(name="dram", bufs=1, space="DRAM") as dp,
        ):
            lst_t = pp.tile([128, K_TOTAL], f16, tag="lst")
            loh_t = pp.tile([128, KBLK * 10], f16, tag="loh")
            nc.sync.dma_start(lst_t[:], lst_d[:])
            nc.sync.dma_start(loh_t[:], loh_d[:])
            if LO_SPLIT:
                lol_t = pp.tile([128, KBLK * 10], f16, tag="lol")
                nc.sync.dma_start(lol_t[:], lol_d[:])

            r_rows = pp.tile([10, NP_CORE], f32, tag="r_rows")

            for c in range(N_CHUNKS):
                f_t = fpool.tile([4 * NST, CHUNK], f16, tag="fchunk")
                nc.sync.dma_start(f_t[:], fst_d[:, c * CHUNK:(c + 1) * CHUNK])
                po = pso.tile([10, CHUNK], f32, tag="po")
                for p in range(KBLK // 2):      # k-block pairs
                    kb0, kb1 = 2 * p, 2 * p + 1
                    ps_s = psps.tile([128, 2 * CHUNK], f32, tag="ps_s")
                    ps_t = pspt.tile([128, 2 * CHUNK], f32, tag="ps_t")
                    # four feature matmuls rotate over the 32-row groups
                    for g, (dst, kb) in enumerate(
                            [(ps_s, kb0), (ps_t, kb0), (ps_s, kb1), (ps_t, kb1)]):
                        half = slice((kb - 2 * p) * CHUNK, (kb - 2 * p + 1) * CHUNK)
                        nc.tensor.matmul(
                            dst[:, half],
                            lst_t[32 * g:32 * (g + 1), kb * 128:(kb + 1) * 128],
                            f_t[32 * g:32 * (g + 1), :],
                            start=True, stop=True,
                            tile_position=(32 * g, 0) if PACK_FEATURES else None)
                    e_t = wp.tile([128, 2 * CHUNK], f32, tag="e")
                    nc.scalar.activation(e_t[:], ps_s[:], AF.Exp, scale=-0.5)
                    w_t = wp.tile([128, 2 * CHUNK], f16, tag="w")
                    nc.vector.tensor_mul(w_t[:], ps_t[:], e_t[:])
                    for kb in (kb0, kb1):
                        half = slice((kb - 2 * p) * CHUNK, (kb - 2 * p + 1) * CHUNK)
                        nc.tensor.matmul(
                            po[:], loh_t[:, kb * 10:(kb + 1) * 10], w_t[:, half],
                            start=(kb == 0),
                            stop=(kb == KBLK - 1 and not LO_SPLIT))
                        if LO_SPLIT:
                            nc.tensor.matmul(
                                po[:], lol_t[:, kb * 10:(kb + 1) * 10],
                                w_t[:, half],
                                start=False, stop=(kb == KBLK - 1))
                # drain R for this chunk to SBUF rows (alternate engines)
                dst = r_rows[:, c * CHUNK:(c + 1) * CHUNK]
                if c % 2 == 0:
                    nc.scalar.copy(dst, po[:])
                else:
                    nc.vector.tensor_copy(dst, po[:])

            # output row 0 = R0 (bias added on host)
            nc.sync.dma_start(out_d[0:1, :], r_rows[0:1, :])

            # epilogue: bounce R1..R9 through DRAM into [128, 64] point layout
            r_dram = dp.tile([10, NP_CORE], f32, tag="r_dram")
            nc.sync.dma_start(r_dram[:], r_rows[:])

            ep_t = []
            for i in range(6):  # x2, x, y2, y, z2, z
                t = pp.tile([128, EPP], f32, tag=f"ep{i}")
                nc.sync.dma_start(
                    t[:], ep_d[i:i + 1, :].rearrange("o (p f) -> (o p) f", p=128))
                ep_t.append(t)

            for j in range(3):  # d2x, d2y, d2z
                r1 = wp.tile([128, EPP], f32, tag="r1")
                r2 = wp.tile([128, EPP], f32, tag="r2")
                r3 = wp.tile([128, EPP], f32, tag="r3")
                for idx, t in zip(range(1 + 3 * j, 4 + 3 * j), (r1, r2, r3)):
                    nc.sync.dma_start(
                        t[:], r_dram[idx:idx + 1, :].rearrange(
                            "o (p f) -> (o p) f", p=128))
                sq_t, lin_t = ep_t[2 * j], ep_t[2 * j + 1]
                m1 = wp.tile([128, EPP], f32, tag="m1")
                nc.vector.tensor_mul(m1[:], sq_t[:], r1[:])
                m2 = wp.tile([128, EPP], f32, tag="m2")
                nc.vector.tensor_mul(m2[:], lin_t[:], r2[:])
                a1 = wp.tile([128, EPP], f32, tag="a1")
                nc.vector.tensor_add(a1[:], m1[:], m2[:])
                d2 = wp.tile([128, EPP], f32, tag="d2")
                nc.vector.tensor_add(d2[:], a1[:], r3[:])
                nc.sync.dma_start(
                    out_d[j + 1:j + 2, :].rearrange("o (p f) -> (o p) f", p=128),
                    d2[:])

    nc.compile()
    return nc


def _get_program():
    if "nc" not in _COMPILED:
        _COMPILED["nc"] = _build_program()
    return _COMPILED["nc"]


def _host_prep(x, y, z, wx, bx, wy, by, wz, bz, coeff):
    """Build per-core input maps (features + coefficient matrices)."""
    f8 = np.float64
    wx64, bx64 = wx.astype(f8), bx.astype(f8)
    wy64, by64 = wy.astype(f8), by.astype(f8)
    wz64, bz64 = wz.astype(f8), bz.astype(f8)
    c64 = coeff.astype(f8)
    sc = np.sqrt(np.clip(wx64 * wy64 * wz64, 1e-12, None))
    Z = np.zeros_like(wx64)

    # s = xt^2 + yt^2 + zt^2 over features [x2,y2,z2,xyz,xy,xz,yz,x,y,z,1]
    Ls = np.stack([
        wx64 ** 2, wy64 ** 2, wz64 ** 2, Z, Z, Z, Z,
        -2 * wx64 * bx64, -2 * wy64 * by64, -2 * wz64 * bz64,
        bx64 ** 2 + by64 ** 2 + bz64 ** 2,
    ]).astype(np.float32)                      # [11, K]
    # T3 = xt*yt*zt
    Lt = np.stack([
        Z, Z, Z,
        wx64 * wy64 * wz64, -wx64 * wy64 * bz64, -wx64 * by64 * wz64,
        -bx64 * wy64 * wz64, wx64 * by64 * bz64, bx64 * wy64 * bz64,
        bx64 * by64 * wz64, -bx64 * by64 * bz64,
    ]).astype(np.float32)                      # [11, K]
    b1 = c64 * sc * wx64 ** 2
    b2 = c64 * sc * wy64 ** 2
    b3 = c64 * sc * wz64 ** 2
    Lo = np.stack([
        -c64 * sc,
        -b1 * wx64 ** 2, 2 * b1 * wx64 * bx64, b1 * (3 - bx64 ** 2),
        -b2 * wy64 ** 2, 2 * b2 * wy64 * by64, b2 * (3 - by64 ** 2),
        -b3 * wz64 ** 2, 2 * b3 * wz64 * bz64, b3 * (3 - bz64 ** 2),
    ], axis=1).astype(np.float32)              # [K, 10]

    Ls32 = _stack32(Ls)                        # [32, K] fp16
    Lt32 = _stack32(Lt)
    lst_pack = np.concatenate([Ls32, Lt32, Ls32, Lt32], axis=0)  # [128, K]
    Loh, Lol = _split16(Lo)
    loh_pack = np.concatenate(
        [Loh[kb * 128:(kb + 1) * 128] for kb in range(KBLK)], axis=1)  # [128, 40]
    lol_pack = np.concatenate(
        [Lol[kb * 128:(kb + 1) * 128] for kb in range(KBLK)], axis=1)

    in_maps = []
    for cid in range(N_CORES):
        sl = slice(cid * NP_CORE, (cid + 1) * NP_CORE)
        xs, ys, zs = (np.ascontiguousarray(a[sl], np.float32) for a in (x, y, z))
        F = np.stack([
            xs * xs, ys * ys, zs * zs, xs * ys * zs, xs * ys, xs * zs,
            ys * zs, xs, ys, zs, np.ones_like(xs),
        ]).astype(np.float32)                  # [11, NP_CORE]
        Fh, Fl = _split16(F)
        f32s = np.concatenate([Fh, Fh, Fl[:NFEAT - 1]], axis=0)   # [32, NP]
        fst = np.concatenate([f32s] * 4, axis=0)                  # [128, NP]
        ep = np.stack([xs * xs, xs, ys * ys, ys, zs * zs, zs]).astype(np.float32)
        in_maps.append({
            "fst": fst, "lst": lst_pack,
            "loh": loh_pack, "lol": lol_pack, "ep": ep,
        })
    return in_maps


def _run_device(in_maps, trace=False):
    from concourse.bass_utils import run_bass_kernel_spmd
    nc = _get_program()
    return run_bass_kernel_spmd(nc, in_maps, list(range(N_CORES)), trace=trace)


def kernel(x, y, z, wx, bx, wy, by, wz, bz, coeff, bias, _trace=False):
    x, y, z = (np.asarray(a, np.float32) for a in (x, y, z))
    in_maps = _host_prep(
        x, y, z,
        *(np.asarray(a, np.float32) for a in (wx, bx, wy, by, wz, bz, coeff)))
    res = _run_device(in_maps, trace=_trace)
    outs = [res.results[cid]["out"] for cid in range(N_CORES)]
    full = np.concatenate(outs, axis=1)        # [4, N_TOTAL]
    bias_f = np.float32(np.asarray(bias))
    output = (full[0] + bias_f).astype(np.float32)
    if _trace:
        kernel._last_results = res
    return (output, full[1].copy(), full[2].copy(), full[3].copy())
